# revision 7
# baseline (speedup 1.0000x reference)
"""Trainium2 Bass kernel: decode-step (paged) attention MLP block, TP over heads on 8 cores.

Contract: kernel(**inputs) takes the FULL inputs (as produced by the problem's
setup_inputs) and returns the FULL [B, HID] output.  Internally: shard
wq/wk/wv columns, wo rows and the KV caches along the head axis across 8
NeuronCores, run a Bass/Tile kernel per core (same program, different data),
all-reduce the wo partial products on device, and return core 0's output.
"""
import sys
import numpy as np

sys.path.insert(0, '/opt/trn_rl_repo')

import concourse.bass as bass
import concourse.bacc as bacc
import concourse.tile as tile
from concourse import mybir
from concourse.masks import make_identity

# Problem constants (hardcoded per contract).
B, HID, H, D = 16, 4096, 32, 128
BS, MB = 16, 64
NB = B * MB
MAXCTX = MB * BS
ROPE_BASE = 10000.0
SCALE = 1.0 / float(np.sqrt(D))
EPS = 1e-5
N_CORES = 8
HPC = H // N_CORES          # heads per core (4)
HD = HPC * D                # head dims per core (512)
F32 = mybir.dt.float32

CHUNK = 128                 # attention token chunk (PE tile)
BPC = CHUNK // BS           # cache blocks per chunk (8)


def _runs(ids):
    """Split a list of block ids into (start, count) runs of consecutive ids."""
    runs = []
    for i, v in enumerate(ids):
        if runs and v == runs[-1][0] + runs[-1][1]:
            runs[-1][1] += 1
        else:
            runs.append([int(v), 1])
    return runs


def build_nc(positions, block_tables):
    """Trace the per-core Bass program.  positions/block_tables are host
    values (np arrays) and are baked into the instruction stream (trip counts
    and DMA descriptors); the numeric tensors arrive via DRAM inputs."""
    pos = np.asarray(positions, dtype=np.int64)
    bt = np.asarray(block_tables, dtype=np.int64)

    nc = bacc.Bacc("TRN2", target_bir_lowering=False, debug=False,
                   enable_asserts=False, num_devices=N_CORES)

    dt = F32
    xT_d = nc.dram_tensor("xT", [HID, B], dt, kind="ExternalInput").ap()
    xadd_d = nc.dram_tensor("xadd", [B, HID], dt, kind="ExternalInput").ap()
    cc2_d = nc.dram_tensor("cc2", [D, B], dt, kind="ExternalInput").ap()
    ss2_d = nc.dram_tensor("ss2", [D, B], dt, kind="ExternalInput").ap()
    rotm_d = nc.dram_tensor("rotm", [D, D], dt, kind="ExternalInput").ap()
    pmask_d = nc.dram_tensor("pmask", [CHUNK, B], dt, kind="ExternalInput").ap()
    wq_d = nc.dram_tensor("wq", [HID, HD], dt, kind="ExternalInput").ap()
    wk_d = nc.dram_tensor("wk", [HID, HD], dt, kind="ExternalInput").ap()
    wv_d = nc.dram_tensor("wv", [HID, HD], dt, kind="ExternalInput").ap()
    wo_d = nc.dram_tensor("wo", [HD, HID], dt, kind="ExternalInput").ap()
    kc_d = nc.dram_tensor("kc", [HPC, NB, BS, D], dt, kind="ExternalInput").ap()
    vc_d = nc.dram_tensor("vc", [HPC, NB, BS, D], dt, kind="ExternalInput").ap()
    y_d = nc.dram_tensor("y", [B, HID], dt, kind="ExternalOutput").ap()

    MH = HID // CHUNK  # 32 hid chunks

    with tile.TileContext(nc) as tc:
        with tc.tile_pool(name="const", bufs=1) as constp, \
             tc.tile_pool(name="persist", bufs=1) as persist, \
             tc.tile_pool(name="wstream", bufs=3) as wstream, \
             tc.tile_pool(name="wbig", bufs=32) as wbig, \
             tc.tile_pool(name="kv", bufs=2) as kvp, \
             tc.tile_pool(name="kt", bufs=2) as ktp, \
             tc.tile_pool(name="probs", bufs=2) as probsp, \
             tc.tile_pool(name="small", bufs=4) as smallp, \
             tc.tile_pool(name="psS", bufs=2, space="PSUM") as psS, \
             tc.tile_pool(name="psT", bufs=2, space="PSUM") as psT, \
             tc.tile_pool(name="psA", bufs=1, space="PSUM") as psA, \
             tc.tile_pool(name="dram", bufs=1, space="DRAM") as dramp:

            ident = constp.tile([128, 128], dt)
            make_identity(nc, ident)
            ones_col = constp.tile([128, 1], dt)
            nc.vector.memset(ones_col, 1.0)
            ones_row = constp.tile([1, 128], dt)
            nc.vector.memset(ones_row, 1.0)
            eps_t = constp.tile([1, 1], dt)
            nc.vector.memset(eps_t, EPS)

            # ---- Phase 1: LN stats from xT chunks (PE ones-reduction) ----
            xT_tiles = []
            sum_ps = psS.tile([1, B], F32, tag="s")
            sq_ps = psS.tile([1, B], F32, tag="s")
            for m in range(MH):
                xm = persist.tile([128, B], dt, tag=f"xT{m}")
                nc.sync.dma_start(out=xm, in_=xT_d[m * 128:(m + 1) * 128, :])
                xT_tiles.append(xm)
                nc.tensor.matmul(sum_ps, ones_col, xm,
                                 start=(m == 0), stop=(m == MH - 1))
            for m in range(MH):
                sq = smallp.tile([128, B], dt, tag="sq")
                nc.vector.tensor_mul(sq, xT_tiles[m], xT_tiles[m])
                nc.tensor.matmul(sq_ps, ones_col, sq,
                                 start=(m == 0), stop=(m == MH - 1))
            mu_row = persist.tile([1, B], dt, tag="mu_row")
            nc.scalar.activation(out=mu_row, in_=sum_ps,
                                 func=mybir.ActivationFunctionType.Copy,
                                 scale=1.0 / HID)
            ex2_row = smallp.tile([1, B], dt, tag="ex2")
            nc.scalar.activation(out=ex2_row, in_=sq_ps,
                                 func=mybir.ActivationFunctionType.Copy,
                                 scale=1.0 / HID)
            var_row = smallp.tile([1, B], dt, tag="var")
            nc.vector.tensor_mul(var_row, mu_row, mu_row)
            nc.vector.tensor_sub(var_row, ex2_row, var_row)
            # rstd = 1/sqrt(var+eps)
            std_row = smallp.tile([1, B], dt, tag="std")
            nc.scalar.activation(out=std_row, in_=var_row,
                                 func=mybir.ActivationFunctionType.Sqrt,
                                 bias=eps_t)
            rstd_row = persist.tile([1, B], dt, tag="rstd_row")
            nc.vector.reciprocal(out=rstd_row, in_=std_row)
            # broadcast mu/rstd across 128 partitions via PE
            mu_ps = psS.tile([128, B], F32, tag="s")
            nc.tensor.matmul(mu_ps, ones_row, mu_row, start=True, stop=True)
            mu_bc = persist.tile([128, B], dt, tag="mu_bc")
            nc.scalar.copy(out=mu_bc, in_=mu_ps)
            rs_ps = psS.tile([128, B], F32, tag="s")
            nc.tensor.matmul(rs_ps, ones_row, rstd_row, start=True, stop=True)
            rs_bc = persist.tile([128, B], dt, tag="rs_bc")
            nc.scalar.copy(out=rs_bc, in_=rs_ps)

            # ---- Phase 2: xnT chunks ----
            xnT_tiles = []
            for m in range(MH):
                xn = persist.tile([128, B], dt, tag=f"xnT{m}")
                nc.vector.tensor_sub(xn, xT_tiles[m], mu_bc)
                nc.vector.tensor_mul(xn, xn, rs_bc)
                xnT_tiles.append(xn)

            # ---- Phase 3: q/k/v projections, transposed [D, B] per head ----
            cc2 = persist.tile([D, B], dt, tag="cc2")
            nc.sync.dma_start(out=cc2, in_=cc2_d)
            ss2 = persist.tile([D, B], dt, tag="ss2")
            nc.sync.dma_start(out=ss2, in_=ss2_d)
            rotm = persist.tile([D, D], dt, tag="rotm")
            nc.sync.dma_start(out=rotm, in_=rotm_d)
            pmask = persist.tile([CHUNK, B], dt, tag="pmask")
            nc.sync.dma_start(out=pmask, in_=pmask_d)

            def rope(dst, src):
                # rot(q) = q*cc2 + (P q)*ss2, with P the signed half-swap
                sw_ps = psS.tile([D, B], F32, tag="s")
                nc.tensor.matmul(sw_ps, rotm, src, start=True, stop=True)
                swp = smallp.tile([D, B], dt, tag="ropeSw")
                nc.scalar.copy(out=swp, in_=sw_ps)
                t1 = smallp.tile([D, B], dt, tag="ropeA")
                t2 = smallp.tile([D, B], dt, tag="ropeB")
                nc.vector.tensor_mul(t1, src, cc2)
                nc.vector.tensor_mul(t2, swp, ss2)
                nc.vector.tensor_add(dst, t1, t2)

            qT, kTn, vTn = [], [], []
            for mat_i, w_d in enumerate((wq_d, wk_d, wv_d)):
                wtiles = []
                for m in range(MH):
                    wm = wbig.tile([128, HD], dt, tag="wmat")
                    nc.sync.dma_start(out=wm, in_=w_d[m * 128:(m + 1) * 128, :])
                    wtiles.append(wm)
                for h in range(HPC):
                    pp = psT.tile([D, B], F32, tag="t")
                    for m in range(MH):
                        nc.tensor.matmul(pp, wtiles[m][:, h * D:(h + 1) * D],
                                         xnT_tiles[m],
                                         start=(m == 0), stop=(m == MH - 1))
                    raw = smallp.tile([D, B], dt, tag="rawproj")
                    nc.scalar.copy(out=raw, in_=pp)
                    if mat_i == 2:
                        v = persist.tile([D, B], dt, tag=f"vT{h}")
                        nc.vector.tensor_copy(out=v, in_=raw)
                        vTn.append(v)
                    else:
                        dstl = qT if mat_i == 0 else kTn
                        rot = persist.tile([D, B], dt, tag=f"rot{mat_i}_{h}")
                        rope(rot, raw)
                        dstl.append(rot)

            # ---- Phase 4: paged attention over the cache, per (h, b) ----
            attn_ps = psA.tile([D, HPC * B], F32)
            dn_rows = []
            for h in range(HPC):
                dnr = persist.tile([1, B], dt, tag=f"dn{h}")
                nc.vector.memset(dnr, 0.0)
                dn_rows.append(dnr)

            for h in range(HPC):
                for b in range(B):
                    p_b = int(pos[b])
                    C = (p_b + CHUNK - 1) // CHUNK
                    if C == 0:
                        continue
                    nblk = C * BPC
                    ids = bt[b, :nblk]
                    knat = kvp.tile([128, C, 128], dt, tag="knat")
                    vnat = kvp.tile([128, C, 128], dt, tag="vnat")
                    for src_d, dst in ((kc_d, knat), (vc_d, vnat)):
                        tok = src_d[h].rearrange("nb t d -> (nb t) d")
                        done = 0
                        for start, cnt in _runs(ids):
                            if done % BPC == 0 and cnt % BPC == 0:
                                src = tok[start * BS:(start + cnt) * BS, :]
                                src = src.rearrange("(c p) d -> p c d", p=CHUNK)
                                nc.sync.dma_start(
                                    out=dst[:, done // BPC:(done + cnt) // BPC, :],
                                    in_=src)
                            else:
                                for j in range(cnt):
                                    g = done + j
                                    p0 = (g % BPC) * BS
                                    nc.sync.dma_start(
                                        out=dst[p0:p0 + BS, g // BPC, :],
                                        in_=tok[(start + j) * BS:(start + j + 1) * BS, :])
                            done += cnt
                    kT = ktp.tile([128, C, 128], dt, tag="kT")
                    for g0 in range(0, C, 4):
                        gn = min(4, C - g0)
                        tp = psT.tile([128, 4 * 128], F32, tag="t")
                        for ci in range(gn):
                            nc.tensor.transpose(
                                tp[:, ci * 128:(ci + 1) * 128],
                                knat[:, g0 + ci, :], ident)
                        nc.scalar.copy(
                            out=kT[:, g0:g0 + gn, :].rearrange("p c d -> p (c d)"),
                            in_=tp[:, 0:gn * 128])
                    probs = probsp.tile([128, C], dt, tag="probs")
                    for c in range(C):
                        lg = psS.tile([128, 1], F32, tag="s")
                        nc.tensor.matmul(lg, kT[:, c, :], qT[h][:, b:b + 1],
                                         start=True, stop=True)
                        nc.scalar.activation(out=probs[:, c:c + 1], in_=lg,
                                             func=mybir.ActivationFunctionType.Exp,
                                             scale=SCALE)
                    rem = p_b - (C - 1) * CHUNK
                    if rem < CHUNK:
                        nc.vector.tensor_mul(probs[:, C - 1:C],
                                             probs[:, C - 1:C],
                                             pmask[:, b:b + 1])
                    for c in range(C):
                        nc.tensor.matmul(attn_ps[:, h * B + b:h * B + b + 1],
                                         vnat[:, c, :], probs[:, c:c + 1],
                                         start=(c == 0), stop=(c == C - 1),
                                         skip_group_check=True)
                    dn = psS.tile([1, C], F32, tag="s")
                    nc.tensor.matmul(dn, ones_col, probs, start=True, stop=True)
                    nc.vector.reduce_sum(out=dn_rows[h][:, b:b + 1], in_=dn,
                                         axis=mybir.AxisListType.X)

            # ---- Phase 5: new token + normalization, per head ----
            attnF = []
            for h in range(HPC):
                prod = smallp.tile([D, B], dt, tag="prod")
                nc.vector.tensor_mul(prod, qT[h], kTn[h])
                ln_ps = psS.tile([1, B], F32, tag="s")
                nc.tensor.matmul(ln_ps, ones_col, prod, start=True, stop=True)
                pnew = smallp.tile([1, B], dt, tag="pnew")
                nc.scalar.activation(out=pnew, in_=ln_ps,
                                     func=mybir.ActivationFunctionType.Exp,
                                     scale=SCALE)
                den = smallp.tile([1, B], dt, tag="den")
                nc.vector.tensor_add(den, dn_rows[h], pnew)
                rec = smallp.tile([1, B], dt, tag="rec")
                nc.vector.reciprocal(out=rec, in_=den)
                pb_ps = psS.tile([128, B], F32, tag="s")
                nc.tensor.matmul(pb_ps, ones_row, pnew, start=True, stop=True)
                pb = smallp.tile([128, B], dt, tag="pb")
                nc.scalar.copy(out=pb, in_=pb_ps)
                rb_ps = psS.tile([128, B], F32, tag="s")
                nc.tensor.matmul(rb_ps, ones_row, rec, start=True, stop=True)
                rb = smallp.tile([128, B], dt, tag="rb")
                nc.scalar.copy(out=rb, in_=rb_ps)
                asb = smallp.tile([D, B], dt, tag="asb")
                nc.scalar.copy(out=asb, in_=attn_ps[:, h * B:(h + 1) * B])
                for b in range(B):
                    if int(pos[b]) == 0:
                        nc.vector.memset(asb[:, b:b + 1], 0.0)
                tmp = smallp.tile([D, B], dt, tag="tmpv")
                nc.vector.tensor_mul(tmp, vTn[h], pb)
                af = persist.tile([D, B], dt, tag=f"attnF{h}")
                nc.vector.tensor_add(af, asb, tmp)
                nc.vector.tensor_mul(af, af, rb)
                attnF.append(af)

            # ---- Phase 6: wo + residual ----
            xadd_sb = persist.tile([B, HID], dt, tag="xadd")
            nc.sync.dma_start(out=xadd_sb, in_=xadd_d)
            y_sb = persist.tile([B, HID], dt, tag="y_sb")
            NJ = HID // 512
            for j in range(NJ):
                yp = psT.tile([B, 512], F32, tag="t")
                for h in range(HPC):
                    wt = wstream.tile([128, 512], dt, tag="wo")
                    nc.sync.dma_start(
                        out=wt, in_=wo_d[h * D:(h + 1) * D, j * 512:(j + 1) * 512])
                    nc.tensor.matmul(yp, attnF[h], wt,
                                     start=(h == 0), stop=(h == HPC - 1))
                nc.vector.tensor_add(y_sb[:, j * 512:(j + 1) * 512], yp,
                                     xadd_sb[:, j * 512:(j + 1) * 512])

            # ---- Phase 7: all-reduce partials, write output ----
            yin = dramp.tile([B, HID], dt)
            yout = dramp.tile([B, HID], dt)
            nc.sync.dma_start(out=yin, in_=y_sb)
            nc.gpsimd.collective_compute(
                "AllReduce", mybir.AluOpType.add,
                replica_groups=[list(range(N_CORES))],
                ins=[yin.opt()], outs=[yout.opt()])
            nc.sync.dma_start(out=y_d, in_=yout)

    nc.compile()
    return nc


def make_in_maps(x, positions, key_cache, value_cache, block_tables,
                 wq, wk, wv, wo):
    x = np.asarray(x, dtype=np.float32)
    pos = np.asarray(positions)
    kcf = np.asarray(key_cache, dtype=np.float32)
    vcf = np.asarray(value_cache, dtype=np.float32)
    wq = np.asarray(wq, dtype=np.float32)
    wk = np.asarray(wk, dtype=np.float32)
    wv = np.asarray(wv, dtype=np.float32)
    wo = np.asarray(wo, dtype=np.float32)

    half = D // 2
    inv_freq = 1.0 / (ROPE_BASE ** (np.arange(half, dtype=np.float32) * 2.0 / D))
    ang = pos.astype(np.float32)[:, None] * inv_freq  # [B, half]
    cosT = np.cos(ang).T.astype(np.float32)   # [half, B]
    sinT = np.sin(ang).T.astype(np.float32)
    cc2 = np.ascontiguousarray(np.concatenate([cosT, cosT], axis=0))  # [D, B]
    ss2 = np.ascontiguousarray(np.concatenate([sinT, sinT], axis=0))
    rotm = np.zeros((D, D), dtype=np.float32)
    for i in range(D // 2):
        rotm[D // 2 + i, i] = -1.0
        rotm[i, D // 2 + i] = 1.0
    pmask = np.zeros((128, B), dtype=np.float32)
    for b in range(B):
        p_b = int(pos[b])
        if p_b > 0:
            rem = p_b - (p_b - 1) // 128 * 128
            pmask[:rem, b] = 1.0
    xT = np.ascontiguousarray(x.T)

    in_maps = []
    for c in range(N_CORES):
        hs = slice(c * HPC, (c + 1) * HPC)
        cs = slice(c * HD, (c + 1) * HD)
        in_maps.append(dict(
            xT=xT,
            xadd=x if c == 0 else np.zeros_like(x),
            cc2=cc2, ss2=ss2, rotm=rotm, pmask=pmask,
            wq=np.ascontiguousarray(wq[:, cs]),
            wk=np.ascontiguousarray(wk[:, cs]),
            wv=np.ascontiguousarray(wv[:, cs]),
            wo=np.ascontiguousarray(wo[cs, :]),
            kc=np.ascontiguousarray(kcf[:, hs].transpose(1, 0, 2, 3)),
            vc=np.ascontiguousarray(vcf[:, hs].transpose(1, 0, 2, 3)),
        ))
    return in_maps


def kernel(x, positions, key_cache, value_cache, block_tables, wq, wk, wv, wo):
    from concourse.bass_utils import run_bass_kernel_spmd
    nc = build_nc(np.asarray(positions), np.asarray(block_tables))
    in_maps = make_in_maps(x, positions, key_cache, value_cache, block_tables,
                           wq, wk, wv, wo)
    res = run_bass_kernel_spmd(nc, in_maps, core_ids=list(range(N_CORES)))
    return res.results[0]["y"].astype(np.float32)
